# revision 9
# baseline (speedup 1.0000x reference)
"""Complex scaled-dot-product attention TRN2 kernel.

Full inputs -> shard (B*H) over 8 cores (2 pairs/core) -> bass SPMD -> gather.
p_attn = softmax(|QK^H|*scale) is the dominant output (256MB fp32).

Math per (b,h) pair:
  sr = Qr@Kr^T + Qi@Ki^T          (one K=128 matmul via d-stacking)
  si = Qr@Ki^T - Qi@Kr^T          (one K=128 matmul vs [Ki; -Kr])
  mag = sqrt(sr^2 + si^2)
  p = softmax(scale * mag)        (no max-subtraction needed: scale*mag <= ~9)
  out_r = p@Vr ; out_i = p@Vi     (bf16 PV with PE-transposed p tiles)
"""

import numpy as np

B, H, S, D = 2, 8, 2048, 64
NCORES = 8
PAIRS = (B * H) // NCORES  # 2

_NC_CACHE = {}


def build_nc(pairs=PAIRS, s=S, d=D, half=4, sr_on_act=True):
    """Build + compile the single-core bass module (SPMD across cores)."""
    from contextlib import ExitStack

    import concourse.bacc as bacc
    import concourse.tile as tile
    from concourse import mybir
    from concourse.masks import make_identity
    from concourse.tile_autobufs import add_dep_helper

    P = 128
    DC = 2 * d                      # 128, d-stacked contraction
    KC = min(512, s)                # score chunk (psum bank)
    NQT = s // P                    # q tiles
    NKC = s // KC                   # score chunks per q tile
    NKT = s // P                    # k subtiles for PV
    half = min(half, NQT)           # q-tiles per table phase block
    scale = float(1.0 / np.sqrt(d))
    f32 = mybir.dt.float32
    f32r = mybir.dt.float32r
    bf16 = mybir.dt.bfloat16
    FT = mybir.ActivationFunctionType

    nc = bacc.Bacc("TRN2", target_bir_lowering=False, debug=False)

    qr = nc.dram_tensor("qr", [pairs, s, d], f32, kind="ExternalInput").ap()
    qi = nc.dram_tensor("qi", [pairs, s, d], f32, kind="ExternalInput").ap()
    kr = nc.dram_tensor("kr", [pairs, s, d], f32, kind="ExternalInput").ap()
    ki = nc.dram_tensor("ki", [pairs, s, d], f32, kind="ExternalInput").ap()
    vr = nc.dram_tensor("vr", [pairs, s, d], f32, kind="ExternalInput").ap()
    vi = nc.dram_tensor("vi", [pairs, s, d], f32, kind="ExternalInput").ap()
    p_out = nc.dram_tensor("p_out", [pairs, s, s], f32, kind="ExternalOutput").ap()
    o_out = nc.dram_tensor("o_out", [pairs, 2, s, d], f32, kind="ExternalOutput").ap()

    with ExitStack() as ctx:
        tc = ctx.enter_context(tile.TileContext(nc))
        consts = ctx.enter_context(tc.tile_pool(name="consts", bufs=1))
        prep = ctx.enter_context(tc.tile_pool(name="prep", bufs=1))
        magp = ctx.enter_context(tc.tile_pool(name="magp", bufs=half + 4))
        wk = ctx.enter_context(tc.tile_pool(name="wk", bufs=4))
        etp = ctx.enter_context(tc.tile_pool(name="etp", bufs=3))
        outp = ctx.enter_context(tc.tile_pool(name="outp", bufs=2))
        pfp = ctx.enter_context(tc.tile_pool(name="pfp", bufs=2))
        pbfp = ctx.enter_context(tc.tile_pool(name="pbfp", bufs=2))
        smp = ctx.enter_context(tc.tile_pool(name="smp", bufs=4))
        psa = ctx.enter_context(tc.tile_pool(name="psa", bufs=2, space="PSUM"))
        pst = ctx.enter_context(tc.tile_pool(name="pst", bufs=2, space="PSUM"))
        pso = ctx.enter_context(tc.tile_pool(name="pso", bufs=2, space="PSUM"))

        last_sqrt = [None]
        last_exp = [None]
        ident = consts.tile([P, P], f32)
        make_identity(nc, ident)
        ident_bf = consts.tile([P, P], bf16)
        make_identity(nc, ident_bf)

        for g in range(pairs):
            # ---------------- prep: transposed, d-stacked Q/K + bf16 Vcat
            QcatT = prep.tile([P, s], f32r, tag="QcatT")   # [dstack, q]
            K1T = prep.tile([P, s], f32r, tag="K1T")       # [Kr;Ki]^T
            K2T = prep.tile([P, s], f32r, tag="K2T")       # [Ki;-Kr]^T
            Vc = prep.tile([P, NKT * P], bf16, tag="Vc")  # [k, (Vr|Vi)] tiles

            QN = magp.tile([P, NQT, DC], f32, tag="mag")
            KN = magp.tile([P, NQT, DC], f32, tag="mag")
            KN2 = magp.tile([P, NQT, DC], f32, tag="mag")
            VN = magp.tile([P, NQT, DC], f32, tag="mag")
            for src, dst in ((qr, QN[:, :, 0:d]), (qi, QN[:, :, d:DC]),
                             (kr, KN[:, :, 0:d]), (ki, KN[:, :, d:DC]),
                             (vr, VN[:, :, 0:d]), (vi, VN[:, :, d:DC])):
                nc.sync.dma_start(out=dst, in_=src[g].rearrange("(t p) d -> p t d", p=P))
            # K2 = [Ki | -Kr] built in natural layout (free-dim moves only)
            nc.vector.tensor_copy(KN2[:, :, 0:d], KN[:, :, d:DC])
            nc.vector.tensor_scalar_mul(KN2[:, :, d:DC], KN[:, :, 0:d], -1.0)

            for t in range(NQT):
                tq = pst.tile([P, KC], f32, tag="tp")
                nc.tensor.transpose(tq[:, 0:P], QN[:, t, :], ident)
                nc.vector.tensor_copy(QcatT[:, t * P:(t + 1) * P], tq[:, 0:P])
                tk = pst.tile([P, KC], f32, tag="tp")
                nc.tensor.transpose(tk[:, 0:P], KN[:, t, :], ident)
                nc.vector.tensor_copy(K1T[:, t * P:(t + 1) * P], tk[:, 0:P])
                tk2 = pst.tile([P, KC], f32, tag="tp")
                nc.tensor.transpose(tk2[:, 0:P], KN2[:, t, :], ident)
                nc.vector.tensor_copy(K2T[:, t * P:(t + 1) * P], tk2[:, 0:P])
                nc.vector.tensor_copy(Vc[:, t * P:(t + 1) * P], VN[:, t, :])

            # ---------------- blocks of `half` q-tiles: phase A (scores+sqrt),
            # then phase B (exp+normalize+PV) — batches ACT table sets.
            # Software-pipelined emission: A(n+1) is emitted before B(n) so
            # next-block matmuls overlap this block's exp/PV stream.
            def emit_A(blk):
                mag_tiles = []
                for qt in range(blk, blk + half):
                    magt = magp.tile([P, s], f32, tag="mag")
                    mag_tiles.append(magt)
                    lhs = QcatT[:, qt * P:(qt + 1) * P]
                    for kc in range(NKC):
                        ks = slice(kc * KC, (kc + 1) * KC)
                        srsi = psa.tile([P, 2 * KC], f32, tag="srsi")
                        nc.tensor.matmul(srsi[:, 0:KC], lhs, K1T[:, ks],
                                         start=True, stop=True)
                        nc.tensor.matmul(srsi[:, KC:2 * KC], lhs, K2T[:, ks],
                                         start=True, stop=True)
                        t12 = wk.tile([P, 2 * KC], f32, tag="t12")
                        nc.scalar.activation(t12, srsi, FT.Square)
                        nc.vector.tensor_add(magt[:, ks], t12[:, 0:KC],
                                             t12[:, KC:2 * KC])
                    sq = nc.scalar.activation(magt, magt, FT.Sqrt)
                    if last_exp[0] is not None:
                        add_dep_helper(sq.ins, last_exp[0].ins, sync=True,
                                       reason="act table: sqrt block after exp block")
                    last_sqrt[0] = sq
                return mag_tiles, last_sqrt[0]

            def emit_B(blk, mag_tiles, blk_sqrt):
                for i, qt in enumerate(range(blk, blk + half)):
                    magt = mag_tiles[i]
                    rs = smp.tile([P, 1], f32, tag="rs")
                    # in-place: mag -> e = exp(scale*mag); rs = rowsum(e)
                    ex = nc.scalar.activation(magt, magt, FT.Exp, scale=scale,
                                              accum_out=rs)
                    if blk_sqrt is not None:
                        add_dep_helper(ex.ins, blk_sqrt.ins, sync=True,
                                       reason="act table: exp block after sqrt block")
                    last_exp[0] = ex
                    rrec = smp.tile([P, 1], f32, tag="rrec")
                    nc.vector.reciprocal(rrec, rs)
                    pf = pfp.tile([P, s], f32, tag="pf")
                    nc.vector.tensor_scalar_mul(pf, magt, rrec)
                    pbf = pbfp.tile([P, s], bf16, tag="pbf")
                    nc.vector.tensor_scalar_mul(pbf, magt, rrec)
                    nc.sync.dma_start(out=p_out[g, qt * P:(qt + 1) * P, :], in_=pf)

                    eT = etp.tile([P, s], bf16, tag="eT")
                    for kb in range(NKC):
                        tp = pst.tile([P, KC], bf16, tag="tp")
                        for j in range(KC // P):
                            kt = kb * (KC // P) + j
                            nc.tensor.transpose(tp[:, j * P:(j + 1) * P],
                                                pbf[:, kt * P:(kt + 1) * P], ident_bf)
                        nc.vector.tensor_copy(eT[:, kb * KC:(kb + 1) * KC], tp)
                    ops = pso.tile([P, DC], f32, tag="ops")
                    for kt in range(NKT):
                        nc.tensor.matmul(ops, eT[:, kt * P:(kt + 1) * P],
                                         Vc[:, kt * P:(kt + 1) * P],
                                         start=(kt == 0), stop=(kt == NKT - 1))
                    of = outp.tile([P, DC], f32, tag="of")
                    nc.vector.tensor_copy(of, ops)
                    nc.sync.dma_start(out=o_out[g, 0, qt * P:(qt + 1) * P, :],
                                      in_=of[:, 0:d])
                    nc.sync.dma_start(out=o_out[g, 1, qt * P:(qt + 1) * P, :],
                                      in_=of[:, d:DC])

            pending = None
            for blk in range(0, NQT, half):
                tiles, blk_sqrt = emit_A(blk)
                if pending is not None:
                    emit_B(*pending)
                pending = (blk, tiles, blk_sqrt)
            emit_B(*pending)

    nc.compile()
    return nc


def _get_nc():
    if "nc" not in _NC_CACHE:
        _NC_CACHE["nc"] = build_nc()
    return _NC_CACHE["nc"]


def kernel(**inputs):
    from concourse import bass_utils

    nc = _get_nc()
    arrs = {k: np.ascontiguousarray(np.asarray(v, dtype=np.float32).reshape(B * H, S, D))
            for k, v in inputs.items()}
    names = {"qr": "Q_real", "qi": "Q_imag", "kr": "K_real",
             "ki": "K_imag", "vr": "V_real", "vi": "V_imag"}
    in_maps = []
    for c in range(NCORES):
        sl = slice(PAIRS * c, PAIRS * (c + 1))
        in_maps.append({k: arrs[v][sl] for k, v in names.items()})
    res = bass_utils.run_bass_kernel_spmd(nc, in_maps, core_ids=list(range(NCORES)))
    p = np.stack([res.results[c]["p_out"] for c in range(NCORES)])
    p_attn = p.reshape(B, H, S, S)
    o = np.stack([res.results[c]["o_out"] for c in range(NCORES)])
    out = o.reshape(B * H, 2, S, D).transpose(1, 0, 2, 3).reshape(2, B, H, S, D)
    return (np.ascontiguousarray(out), np.ascontiguousarray(p_attn))


# revision 11
# speedup vs baseline: 1.0457x; 1.0457x over previous
"""Complex scaled-dot-product attention TRN2 kernel.

Full inputs -> shard (B*H) over 8 cores (2 pairs/core) -> bass SPMD -> gather.
p_attn = softmax(|QK^H|*scale) is the dominant output (256MB fp32).

Math per (b,h) pair:
  sr = Qr@Kr^T + Qi@Ki^T          (one K=128 matmul via d-stacking)
  si = Qr@Ki^T - Qi@Kr^T          (one K=128 matmul vs [Ki; -Kr])
  mag = sqrt(sr^2 + si^2)
  p = softmax(scale * mag)        (no max-subtraction needed: scale*mag <= ~9)
  out_r = p@Vr ; out_i = p@Vi     (bf16 PV with PE-transposed p tiles)
"""

import numpy as np

B, H, S, D = 2, 8, 2048, 64
NCORES = 8
PAIRS = (B * H) // NCORES  # 2

_NC_CACHE = {}


def build_nc(pairs=PAIRS, s=S, d=D, half=4, sr_on_act=True):
    """Build + compile the single-core bass module (SPMD across cores)."""
    from contextlib import ExitStack

    import concourse.bacc as bacc
    import concourse.tile as tile
    from concourse import mybir
    from concourse.masks import make_identity
    from concourse.tile_autobufs import add_dep_helper

    P = 128
    DC = 2 * d                      # 128, d-stacked contraction
    KC = min(512, s)                # score chunk (psum bank)
    NQT = s // P                    # q tiles
    NKC = s // KC                   # score chunks per q tile
    NKT = s // P                    # k subtiles for PV
    half = min(half, NQT)           # q-tiles per table phase block
    scale = float(1.0 / np.sqrt(d))
    f32 = mybir.dt.float32
    f32r = mybir.dt.float32r
    bf16 = mybir.dt.bfloat16
    FT = mybir.ActivationFunctionType

    nc = bacc.Bacc("TRN2", target_bir_lowering=False, debug=False)

    qr = nc.dram_tensor("qr", [pairs, s, d], f32, kind="ExternalInput").ap()
    qi = nc.dram_tensor("qi", [pairs, s, d], f32, kind="ExternalInput").ap()
    kr = nc.dram_tensor("kr", [pairs, s, d], f32, kind="ExternalInput").ap()
    ki = nc.dram_tensor("ki", [pairs, s, d], f32, kind="ExternalInput").ap()
    vr = nc.dram_tensor("vr", [pairs, s, d], f32, kind="ExternalInput").ap()
    vi = nc.dram_tensor("vi", [pairs, s, d], f32, kind="ExternalInput").ap()
    p_out = nc.dram_tensor("p_out", [pairs, s, s], f32, kind="ExternalOutput").ap()
    o_out = nc.dram_tensor("o_out", [pairs, 2, s, d], f32, kind="ExternalOutput").ap()

    with ExitStack() as ctx:
        tc = ctx.enter_context(tile.TileContext(nc))
        consts = ctx.enter_context(tc.tile_pool(name="consts", bufs=1))
        prep = ctx.enter_context(tc.tile_pool(name="prep", bufs=1))
        magp = ctx.enter_context(tc.tile_pool(name="magp", bufs=half + 6))
        wk = ctx.enter_context(tc.tile_pool(name="wk", bufs=6))
        etp = ctx.enter_context(tc.tile_pool(name="etp", bufs=4))
        outp = ctx.enter_context(tc.tile_pool(name="outp", bufs=2))
        pfp = ctx.enter_context(tc.tile_pool(name="pfp", bufs=3))
        pbfp = ctx.enter_context(tc.tile_pool(name="pbfp", bufs=2))
        smp = ctx.enter_context(tc.tile_pool(name="smp", bufs=4))
        psa = ctx.enter_context(tc.tile_pool(name="psa", bufs=2, space="PSUM"))
        pst = ctx.enter_context(tc.tile_pool(name="pst", bufs=2, space="PSUM"))
        pso = ctx.enter_context(tc.tile_pool(name="pso", bufs=2, space="PSUM"))

        last_sqrt = [None]
        last_exp = [None]
        ident = consts.tile([P, P], f32)
        make_identity(nc, ident)
        ident_bf = consts.tile([P, P], bf16)
        make_identity(nc, ident_bf)

        for g in range(pairs):
            # ---------------- prep: transposed, d-stacked Q/K + bf16 Vcat
            QcatT = prep.tile([P, s], f32r, tag="QcatT")   # [dstack, q]
            K1T = prep.tile([P, s], f32r, tag="K1T")       # [Kr;Ki]^T
            K2T = prep.tile([P, s], f32r, tag="K2T")       # [Ki;-Kr]^T
            Vc = prep.tile([P, NKT * P], bf16, tag="Vc")  # [k, (Vr|Vi)] tiles

            QN = magp.tile([P, NQT, DC], f32, tag="mag")
            KN = magp.tile([P, NQT, DC], f32, tag="mag")
            KN2 = magp.tile([P, NQT, DC], f32, tag="mag")
            VN = magp.tile([P, NQT, DC], f32, tag="mag")
            for src, dst in ((qr, QN[:, :, 0:d]), (qi, QN[:, :, d:DC]),
                             (kr, KN[:, :, 0:d]), (ki, KN[:, :, d:DC]),
                             (vr, VN[:, :, 0:d]), (vi, VN[:, :, d:DC])):
                nc.sync.dma_start(out=dst, in_=src[g].rearrange("(t p) d -> p t d", p=P))
            # K2 = [Ki | -Kr] built in natural layout (free-dim moves only)
            nc.vector.tensor_copy(KN2[:, :, 0:d], KN[:, :, d:DC])
            nc.vector.tensor_scalar_mul(KN2[:, :, d:DC], KN[:, :, 0:d], -1.0)

            for t in range(NQT):
                tq = pst.tile([P, KC], f32, tag="tp")
                nc.tensor.transpose(tq[:, 0:P], QN[:, t, :], ident)
                nc.vector.tensor_copy(QcatT[:, t * P:(t + 1) * P], tq[:, 0:P])
                tk = pst.tile([P, KC], f32, tag="tp")
                nc.tensor.transpose(tk[:, 0:P], KN[:, t, :], ident)
                nc.vector.tensor_copy(K1T[:, t * P:(t + 1) * P], tk[:, 0:P])
                tk2 = pst.tile([P, KC], f32, tag="tp")
                nc.tensor.transpose(tk2[:, 0:P], KN2[:, t, :], ident)
                nc.vector.tensor_copy(K2T[:, t * P:(t + 1) * P], tk2[:, 0:P])
                nc.vector.tensor_copy(Vc[:, t * P:(t + 1) * P], VN[:, t, :])

            # ---------------- blocks of `half` q-tiles: phase A (scores+sqrt),
            # then phase B (exp+normalize+PV) — batches ACT table sets.
            for blk in range(0, NQT, half):
                mag_tiles = []
                for qt in range(blk, blk + half):
                    magt = magp.tile([P, s], f32, tag="mag")
                    mag_tiles.append(magt)
                    lhs = QcatT[:, qt * P:(qt + 1) * P]
                    for kc in range(NKC):
                        ks = slice(kc * KC, (kc + 1) * KC)
                        srsi = psa.tile([P, 2 * KC], f32, tag="srsi")
                        nc.tensor.matmul(srsi[:, 0:KC], lhs, K1T[:, ks],
                                         start=True, stop=True)
                        nc.tensor.matmul(srsi[:, KC:2 * KC], lhs, K2T[:, ks],
                                         start=True, stop=True)
                        t12 = wk.tile([P, 2 * KC], f32, tag="t12")
                        nc.scalar.activation(t12, srsi, FT.Square)
                        nc.vector.tensor_add(magt[:, ks], t12[:, 0:KC],
                                             t12[:, KC:2 * KC])
                    sq = nc.scalar.activation(magt, magt, FT.Sqrt)
                    if last_exp[0] is not None:
                        add_dep_helper(sq.ins, last_exp[0].ins, sync=True,
                                       reason="act table: sqrt block after exp block")
                    last_sqrt[0] = sq

                for i, qt in enumerate(range(blk, blk + half)):
                    magt = mag_tiles[i]
                    rs = smp.tile([P, 1], f32, tag="rs")
                    # in-place: mag -> e = exp(scale*mag); rs = rowsum(e)
                    ex = nc.scalar.activation(magt, magt, FT.Exp, scale=scale,
                                              accum_out=rs)
                    if last_sqrt[0] is not None:
                        add_dep_helper(ex.ins, last_sqrt[0].ins, sync=True,
                                       reason="act table: exp block after sqrt block")
                    last_exp[0] = ex
                    rrec = smp.tile([P, 1], f32, tag="rrec")
                    nc.vector.reciprocal(rrec, rs)
                    pf = pfp.tile([P, s], f32, tag="pf")
                    nc.vector.tensor_scalar_mul(pf, magt, rrec)
                    pbf = pbfp.tile([P, s], bf16, tag="pbf")
                    nc.vector.tensor_scalar_mul(pbf, magt, rrec)
                    nc.sync.dma_start(out=p_out[g, qt * P:(qt + 1) * P, :], in_=pf)

                    eT = etp.tile([P, s], bf16, tag="eT")
                    for kb in range(NKC):
                        tp = pst.tile([P, KC], bf16, tag="tp")
                        for j in range(KC // P):
                            kt = kb * (KC // P) + j
                            nc.tensor.transpose(tp[:, j * P:(j + 1) * P],
                                                pbf[:, kt * P:(kt + 1) * P], ident_bf)
                        nc.vector.tensor_copy(eT[:, kb * KC:(kb + 1) * KC], tp)
                    ops = pso.tile([P, DC], f32, tag="ops")
                    for kt in range(NKT):
                        nc.tensor.matmul(ops, eT[:, kt * P:(kt + 1) * P],
                                         Vc[:, kt * P:(kt + 1) * P],
                                         start=(kt == 0), stop=(kt == NKT - 1))
                    of = outp.tile([P, DC], f32, tag="of")
                    nc.vector.tensor_copy(of, ops)
                    nc.sync.dma_start(out=o_out[g, 0, qt * P:(qt + 1) * P, :],
                                      in_=of[:, 0:d])
                    nc.sync.dma_start(out=o_out[g, 1, qt * P:(qt + 1) * P, :],
                                      in_=of[:, d:DC])

    nc.compile()
    return nc


def _get_nc():
    if "nc" not in _NC_CACHE:
        _NC_CACHE["nc"] = build_nc()
    return _NC_CACHE["nc"]


def kernel(**inputs):
    from concourse import bass_utils

    nc = _get_nc()
    arrs = {k: np.ascontiguousarray(np.asarray(v, dtype=np.float32).reshape(B * H, S, D))
            for k, v in inputs.items()}
    names = {"qr": "Q_real", "qi": "Q_imag", "kr": "K_real",
             "ki": "K_imag", "vr": "V_real", "vi": "V_imag"}
    in_maps = []
    for c in range(NCORES):
        sl = slice(PAIRS * c, PAIRS * (c + 1))
        in_maps.append({k: arrs[v][sl] for k, v in names.items()})
    res = bass_utils.run_bass_kernel_spmd(nc, in_maps, core_ids=list(range(NCORES)))
    p = np.stack([res.results[c]["p_out"] for c in range(NCORES)])
    p_attn = p.reshape(B, H, S, S)
    o = np.stack([res.results[c]["o_out"] for c in range(NCORES)])
    out = o.reshape(B * H, 2, S, D).transpose(1, 0, 2, 3).reshape(2, B, H, S, D)
    return (np.ascontiguousarray(out), np.ascontiguousarray(p_attn))
